# revision 9
# baseline (speedup 1.0000x reference)
"""BurstGNN Trainium2 kernel — single SPMD launch on 8 NeuronCores.

Graph partitioning per the sharding hint: nodes/edges are partitioned by dst
across the 8 cores; the small weights are replicated. All heavy compute runs
on device in ONE launch:

  encoder (feature-major matmuls) -> per-core [x | al | ar] slice
  -> AllGather (halo exchange)    -> full node table in DRAM (bf16)
  -> FAConv layer 1: per-edge source rows fetched with indirect-DMA gathers
     from the table, alpha = tanh(al_src + ar_dst) * norm computed on device,
     scatter-sum via one-hot selection matmuls accumulating in PSUM
  -> AllGather x1 -> FAConv layer 2 -> |.|-smoothing -> AllGather x2
  -> ragged per-user segment sums (one-hot matmuls) -> 2-layer MLP -> logits.

The host only slots edges/users into fixed-depth windows (pure numpy index
arithmetic) and uploads ~4.5 MB per core, instead of pre-gathering ~100 MB
per core of edge streams. Everything crossing cores moves over NeuronLink.
"""

import os
import sys

sys.path.insert(0, "/opt/trn_rl_repo")
os.environ.setdefault("JAX_COMPILATION_CACHE_DIR", "/tmp/jax_cache")

import ml_dtypes
import numpy as np

import concourse.bass as bass
import concourse.bacc as bacc
import concourse.mybir as mybir
import concourse.tile as tile
from concourse.bass import ds

F32 = mybir.dt.float32
BF16 = mybir.dt.bfloat16
I32 = mybir.dt.int32
AF = mybir.ActivationFunctionType
OP = mybir.AluOpType
AX = mybir.AxisListType

EPS = 0.1
LRELU_SLOPE = 0.01


class Cfg:
    def __init__(self, N=200000, E=1600000, U=20000, NUMP=20, CATP=12):
        self.N, self.E, self.U = N, E, U
        self.NUMP, self.CATP = NUMP, CATP
        self.F = NUMP + CATP
        self.C = 8
        self.D = 64
        self.WJ = 32          # dst window width (one-hot width)
        self.GRP = 16         # windows per group -> 512 dst rows / group
        self.NS = N // self.C
        span = self.WJ * self.GRP
        self.NSP = ((self.NS + span - 1) // span) * span
        self.W = self.NSP // self.WJ
        self.G = self.W // self.GRP
        self.TBLR = self.C * self.NSP
        self.UPCU = U // self.C
        self.UW = (self.UPCU + 127) // 128
        self.UPC = self.UW * 128


def _fap(base, dims, extra_off=0):
    return bass.AP(base.tensor, base.offset + extra_off,
                   [list(base.ap[0])] + [list(d) for d in dims])


# --------------------------------------------------------------------------
# Host preprocessing: edge/user slotting (index arithmetic only)
# --------------------------------------------------------------------------

def preprocess(inputs, cfg):
    c = cfg
    src = np.asarray(inputs["edge_index"][0], dtype=np.int64)
    dst = np.asarray(inputs["edge_index"][1], dtype=np.int64)
    offs = np.asarray(inputs["tweet_offsets"], dtype=np.int64)
    re_index = np.asarray(inputs["re_index"], dtype=np.int64)

    deg = np.bincount(dst, minlength=c.N).astype(np.float64) + 1.0
    dinv = (deg ** -0.5).astype(np.float32)

    srcA = np.concatenate([src, np.arange(c.N, dtype=np.int64)])
    dstA = np.concatenate([dst, np.arange(c.N, dtype=np.int64)])
    normA = dinv[srcA] * dinv[dstA]

    core = dstA // c.NS
    dl = dstA - core * c.NS
    wloc = dl // c.WJ
    jloc = (dl - wloc * c.WJ).astype(np.float32)
    gwin = core * c.W + wloc

    cnt = np.bincount(gwin, minlength=c.C * c.W)
    T = max(1, int(-(-cnt.max() // 128)))
    K = c.GRP * T

    order = np.argsort(gwin, kind="stable")
    starts = np.zeros(c.C * c.W + 1, np.int64)
    np.cumsum(cnt, out=starts[1:])
    ranks = np.arange(len(gwin), dtype=np.int64) - starts[gwin[order]]
    t_ = ranks // 128
    p_ = ranks - t_ * 128
    wo = wloc[order]
    k_ = (wo % c.GRP) * T + t_
    g_ = wo // c.GRP
    co = core[order]
    flat = (g_ * 128 + p_) * K + k_

    sz = c.G * 128 * K
    meta_dl = np.full((c.C, sz), -1.0, np.float32)
    slot_norm = np.zeros((c.C, sz), np.float32)
    # pads gather the (all-zero) last pad row of core C-1
    slot_row = np.full((c.C, sz), c.TBLR - 1, np.int32)
    rowidx = ((srcA // c.NS) * c.NSP + (srcA % c.NS)).astype(np.int32)
    meta_dl[co, flat] = jloc[order]
    slot_norm[co, flat] = normA[order]
    slot_row[co, flat] = rowidx[order]

    meta = np.concatenate(
        [meta_dl.reshape(c.C, c.G * 128, K), slot_norm.reshape(c.C, c.G * 128, K)],
        axis=2)                                    # [C, G*128, 2K]
    erow = slot_row.reshape(c.C, c.G * 128, K)     # [C, G*128, K]

    # ---- user phase ----
    st = offs[re_index]
    ln = (offs[re_index + 1] - st).astype(np.int64)
    tot = int(ln.sum())
    uu = np.repeat(np.arange(c.U, dtype=np.int64), ln)
    csl = np.cumsum(ln) - ln
    pos = np.arange(tot, dtype=np.int64) - np.repeat(csl, ln)
    nodes = np.repeat(st, ln) + pos
    ucore = uu // c.UPCU
    ulocal = uu - ucore * c.UPCU
    uw = ulocal // 128
    uj = (ulocal - uw * 128).astype(np.float32)
    guw = ucore * c.UW + uw
    ucnt = np.bincount(guw, minlength=c.C * c.UW)
    KU = max(1, int(-(-ucnt.max() // 128)))

    ustarts = np.zeros(c.C * c.UW + 1, np.int64)
    np.cumsum(ucnt, out=ustarts[1:])
    uranks = np.arange(tot, dtype=np.int64) - ustarts[guw]
    ut = uranks // 128
    up = uranks - ut * 128
    uflat = (uw * 128 + up) * KU + ut

    usz = c.UW * 128 * KU
    umeta = np.full((c.C, usz), -1.0, np.float32)
    urow = np.full((c.C, usz), c.TBLR - 1, np.int32)
    urowidx = ((nodes // c.NS) * c.NSP + (nodes % c.NS)).astype(np.int32)
    umeta[ucore, uflat] = uj
    urow[ucore, uflat] = urowidx
    umeta = umeta.reshape(c.C, c.UW * 128, KU)
    urow = urow.reshape(c.C, c.UW * 128, KU)

    # ---- transposed features per core (pad cols zero) ----
    feat = np.concatenate([np.asarray(inputs["num_prop"], np.float32),
                           np.asarray(inputs["cat_prop"], np.float32)], axis=1)
    featT = np.zeros((c.C, c.F, c.NSP), np.float32)
    for cc in range(c.C):
        featT[cc, :, :c.NS] = feat[cc * c.NS:(cc + 1) * c.NS].T

    bf = ml_dtypes.bfloat16
    return dict(T=T, K=K, KU=KU, meta=meta.astype(bf), erow=erow,
                umeta=umeta.astype(bf), urow=urow, featT=featT.astype(bf))


def make_weights(inputs, cfg):
    c = cfg
    f32 = lambda k: np.asarray(inputs[k], np.float32)
    w1 = np.zeros((c.F, 64), np.float32)
    w1[:c.NUMP, :32] = f32("W_num")
    w1[c.NUMP:, 32:] = f32("W_cat")
    b1 = np.concatenate([f32("b_num"), f32("b_cat")]).reshape(64, 1)
    attlr = np.stack([f32("att_l"), f32("att_r")], axis=1)          # [64, 2]
    attB = np.tile(np.concatenate([f32("att_l"), f32("att_r")])[None, :],
                   (128, 1))                                        # [128, 128]
    bf = ml_dtypes.bfloat16
    return {
        "w1": np.ascontiguousarray(w1).astype(bf),
        "b1": np.ascontiguousarray(b1),
        "wtog": f32("W_tog").astype(bf),
        "btog": f32("b_tog").reshape(64, 1),
        "attlr": np.ascontiguousarray(attlr).astype(bf),
        "attB": np.ascontiguousarray(attB),
        "wf1": f32("W_f1"),
        "bf1": f32("b_f1").reshape(32, 1),
        "wlab": f32("W_lab"),
        "blab": f32("b_lab").reshape(2, 1),
        "iota32": np.tile(np.arange(32, dtype=bf)[None, :], (128, 1)),
        "iota128": np.tile(np.arange(128, dtype=np.float32)[None, :],
                           (128, 1)).astype(bf),
        "ident": np.eye(128, dtype=np.float32),
    }


# --------------------------------------------------------------------------
# Bass program
# --------------------------------------------------------------------------

def build_program(cfg, T, KU):
    c = cfg
    K = c.GRP * T
    T4 = 4 * T
    nc = bacc.Bacc()

    featT_p = nc.declare_dram_parameter("featT", [c.F, c.NSP], BF16, isOutput=False)
    meta_p = nc.declare_dram_parameter("meta", [c.G * 128, 2 * K], BF16, isOutput=False)
    erow_p = nc.declare_dram_parameter("erow", [c.G * 128, K], I32, isOutput=False)
    umeta_p = nc.declare_dram_parameter("umeta", [c.UW * 128, KU], BF16, isOutput=False)
    urow_p = nc.declare_dram_parameter("urow", [c.UW * 128, KU], I32, isOutput=False)
    wparams = {}
    BF_W = {"w1", "wtog", "attlr", "iota32", "iota128"}
    for name, shape in [("w1", [c.F, 64]), ("b1", [64, 1]),
                        ("wtog", [64, 64]), ("btog", [64, 1]),
                        ("attlr", [64, 2]), ("attB", [128, 128]),
                        ("wf1", [64, 32]), ("bf1", [32, 1]),
                        ("wlab", [32, 2]), ("blab", [2, 1]),
                        ("iota32", [128, 32]), ("iota128", [128, 128]),
                        ("ident", [128, 128])]:
        wparams[name] = nc.declare_dram_parameter(
            name, shape, BF16 if name in BF_W else F32, isOutput=False)
    out_p = nc.declare_dram_parameter("out", [2, c.UPC], F32, isOutput=True)

    slice1 = nc.dram_tensor("slice1", [c.NSP, 66], BF16)
    slice2 = nc.dram_tensor("slice2", [c.NSP, 66], BF16)
    slice3 = nc.dram_tensor("slice3", [c.NSP, 64], BF16)
    table1 = nc.dram_tensor("table1", [c.TBLR, 66], BF16, addr_space="Shared")
    table2 = nc.dram_tensor("table2", [c.TBLR, 66], BF16, addr_space="Shared")
    table3 = nc.dram_tensor("table3", [c.TBLR, 64], BF16, addr_space="Shared")

    rg = [list(range(c.C))]

    with tile.TileContext(nc) as tc:
        with tc.tile_pool(name="consts", bufs=1) as cp:
            ws = {}
            for name, p in wparams.items():
                t = cp.tile(list(p.shape), BF16 if name in BF_W else F32, tag=name)
                nc.sync.dma_start(out=t[:], in_=p[:, :])
                ws[name] = t
            ones1 = cp.tile([1, 128], F32)
            nc.vector.memset(ones1[:], 1.0)
            identb = cp.tile([128, 128], BF16)
            nc.vector.tensor_copy(out=identb[:], in_=ws["ident"][:, :])
            beps = cp.tile([128, 1], F32)
            nc.vector.memset(beps[:], 1e-8)
            la = cp.tile([128, K, 128], BF16)
            nc.vector.memset(la[:].rearrange("p k f -> p (k f)"), 0.0)

            # ---------------- encoder ----------------
            def enc_body(ep, epp, t0, pad_from=None):
                ft = ep.tile([c.F, 512], BF16, tag="ft")
                nc.sync.dma_start(out=ft[:], in_=featT_p[:, ds(t0, 512)])
                ps1 = epp.tile([64, 512], F32, tag="ps1")
                nc.tensor.matmul(out=ps1[:], lhsT=ws["w1"][:], rhs=ft[:],
                                 start=True, stop=True)
                mid = ep.tile([64, 512], BF16, tag="mid")
                nc.scalar.activation(out=mid[:], in_=ps1[:], func=AF.Identity,
                                     bias=ws["b1"][:, 0:1])
                nc.vector.scalar_tensor_tensor(out=mid[:], in0=mid[:],
                                               scalar=LRELU_SLOPE, in1=mid[:],
                                               op0=OP.mult, op1=OP.max)
                ps2 = epp.tile([64, 512], F32, tag="ps2")
                nc.tensor.matmul(out=ps2[:], lhsT=ws["wtog"][:], rhs=mid[:],
                                 start=True, stop=True)
                xc = ep.tile([66, 512], BF16, tag="xc")
                nc.scalar.activation(out=xc[0:64, :], in_=ps2[:], func=AF.Identity,
                                     bias=ws["btog"][:, 0:1])
                nc.vector.scalar_tensor_tensor(out=xc[0:64, :], in0=xc[0:64, :],
                                               scalar=LRELU_SLOPE, in1=xc[0:64, :],
                                               op0=OP.mult, op1=OP.max)
                if pad_from is not None and pad_from < 512:
                    nc.vector.memset(xc[0:64, pad_from:512], 0.0)
                ps3 = epp.tile([2, 512], F32, tag="ps3")
                nc.tensor.matmul(out=ps3[:], lhsT=ws["attlr"][:], rhs=xc[0:64, :],
                                 start=True, stop=True)
                nc.scalar.copy(out=xc[64:66, :], in_=ps3[:])
                if pad_from is not None and pad_from < 512:
                    nc.vector.memset(xc[64:66, pad_from:512], 0.0)
                for cc4 in range(4):
                    tp = epp.tile([128, 66], BF16, tag="tp")
                    nc.tensor.transpose(out=tp[:], in_=xc[:, cc4 * 128:(cc4 + 1) * 128],
                                        identity=identb[0:66, 0:66])
                    pb = ep.tile([128, 66], BF16, tag="pb")
                    nc.scalar.copy(out=pb[:], in_=tp[:])
                    nc.sync.dma_start(out=slice1[ds(t0 + cc4 * 128, 128), :],
                                      in_=pb[:])

            with tc.tile_pool(name="enc", bufs=2) as ep, \
                 tc.tile_pool(name="encps", bufs=1, space="PSUM") as epp:
                n_full = c.NSP // 512 - 1
                if n_full > 0:
                    with tc.For_i(0, n_full * 512, 512) as t0:
                        enc_body(ep, epp, t0)
                last0 = n_full * 512
                enc_body(ep, epp, last0, pad_from=c.NS - last0)

            nc.gpsimd.collective_compute(
                "AllGather", OP.bypass, replica_groups=rg,
                ins=[slice1[:, :]], outs=[table1[:, :]])

            # ---------------- FAConv layers ----------------
            def layer_body(lp, lpp, g, src_slice, src_table, layer):
                mfa = lp.tile([128, 2 * K], BF16, tag="mfa")
                nc.sync.dma_start(out=mfa[:], in_=meta_p[ds(g * 128, 128), :])
                ger = lp.tile([128, K], I32, tag="ger")
                nc.sync.dma_start(out=ger[:], in_=erow_p[ds(g * 128, 128), :])
                hg = lp.tile([128, K, 66], BF16, tag="hg")
                for k in range(K):
                    nc.gpsimd.indirect_dma_start(
                        out=hg[:, k, :], out_offset=None,
                        in_=src_table[:, :],
                        in_offset=bass.IndirectOffsetOnAxis(ap=ger[:, k:k + 1],
                                                            axis=0))
                for h in range(4):
                    rb = g * 512 + h * 128
                    kb = h * T4
                    x0b = lp.tile([128, 64], BF16, tag="x0b")
                    nc.sync.dma_start(out=x0b[:], in_=slice1[ds(rb, 128), 0:64])
                    arc = lp.tile([128, 1], BF16, tag="arc")
                    nc.sync.dma_start(out=arc[:], in_=src_slice[ds(rb, 128), 65:66])
                    artp = lpp.tile([1, 128], BF16, tag="artp")
                    nc.tensor.transpose(out=artp[:], in_=arc[:],
                                        identity=identb[:])
                    arsh = lp.tile([1, 128], F32, tag="arsh")
                    nc.scalar.copy(out=arsh[:], in_=artp[:])
                    arw = lpp.tile([128, T4 * 32], F32, tag="arw")
                    nc.tensor.matmul(
                        out=arw[:], lhsT=ones1[:],
                        rhs=_fap(arsh[:], [[32, 4], [0, T], [1, 32]]),
                        start=True, stop=True)
                    addt = lp.tile([128, T4 * 32], F32, tag="addt")
                    nc.vector.tensor_tensor(
                        out=addt[:],
                        in0=_fap(hg[:].rearrange("p k f -> p (k f)"),
                                 [[66, T4], [0, 32]], extra_off=kb * 66 + 64),
                        in1=arw[:], op=OP.add)
                    tanhb = lp.tile([128, T4 * 32], BF16, tag="tanhb")
                    nc.scalar.activation(out=tanhb[:], in_=addt[:], func=AF.Tanh)
                    alph = lp.tile([128, T4 * 32], BF16, tag="alph")
                    nc.vector.tensor_tensor(
                        out=alph[:], in0=tanhb[:],
                        in1=_fap(mfa[:], [[1, T4], [0, 32]], extra_off=K + kb),
                        op=OP.mult)
                    m01 = lp.tile([128, T4 * 32], BF16, tag="m01")
                    nc.vector.tensor_tensor(
                        out=m01[:],
                        in0=_fap(mfa[:], [[1, T4], [0, 32]], extra_off=kb),
                        in1=_fap(ws["iota32"][:], [[0, T4], [1, 32]]),
                        op=OP.is_equal)
                    nc.vector.tensor_tensor(
                        out=_fap(la[:].rearrange("p k f -> p (k f)"),
                                 [[T * 128 + 32, 4], [128, T], [1, 32]],
                                 extra_off=kb * 128),
                        in0=m01[:], in1=alph[:], op=OP.mult)
                    ps = lpp.tile([128, 64], F32, tag="ps")
                    for kk in range(T4):
                        k = kb + kk
                        nc.tensor.matmul(out=ps[:], lhsT=la[:, k, :],
                                         rhs=hg[:, k, 0:64],
                                         start=(kk == 0), stop=(kk == T4 - 1))
                    xo = lp.tile([128, 64], F32, tag="xo")
                    nc.vector.scalar_tensor_tensor(
                        out=xo[:], in0=x0b[:], scalar=EPS, in1=ps[:],
                        op0=OP.mult, op1=OP.add)
                    if layer == 1:
                        pack = lp.tile([128, 66], BF16, tag="pack")
                        nc.scalar.copy(out=pack[:, 0:64], in_=xo[:])
                        palr = lp.tile([128, 2], F32, tag="palr")
                        tsc = lp.tile([128, 64], F32, tag="tsc")
                        nc.vector.tensor_tensor(out=tsc[:], in0=xo[:],
                                                in1=ws["attB"][:, 0:64], op=OP.mult)
                        nc.vector.tensor_reduce(out=palr[:, 0:1], in_=tsc[:],
                                                axis=AX.X, op=OP.add)
                        tsc2 = lp.tile([128, 64], F32, tag="tsc2")
                        nc.vector.tensor_tensor(out=tsc2[:], in0=xo[:],
                                                in1=ws["attB"][:, 64:128], op=OP.mult)
                        nc.vector.tensor_reduce(out=palr[:, 1:2], in_=tsc2[:],
                                                axis=AX.X, op=OP.add)
                        nc.scalar.copy(out=pack[:, 64:66], in_=palr[:])
                        nc.sync.dma_start(out=slice2[ds(rb, 128), :], in_=pack[:])
                    else:
                        xs = lp.tile([128, 64], F32, tag="xs")
                        nc.scalar.activation(out=xs[:], in_=xo[:], func=AF.Square)
                        pack2 = lp.tile([128, 64], BF16, tag="pack2")
                        nc.scalar.activation(out=pack2[:], in_=xs[:], func=AF.Sqrt,
                                             bias=beps[:, 0:1])
                        nc.sync.dma_start(out=slice3[ds(rb, 128), :], in_=pack2[:])

            with tc.tile_pool(name="lay1", bufs=2) as lp, \
                 tc.tile_pool(name="lay1ps", bufs=2, space="PSUM") as lpp:
                with tc.For_i(0, c.G, 1) as g:
                    layer_body(lp, lpp, g, slice1, table1, 1)

            nc.gpsimd.collective_compute(
                "AllGather", OP.bypass, replica_groups=rg,
                ins=[slice2[:, :]], outs=[table2[:, :]])

            with tc.tile_pool(name="lay2", bufs=2) as lp, \
                 tc.tile_pool(name="lay2ps", bufs=2, space="PSUM") as lpp:
                with tc.For_i(0, c.G, 1) as g:
                    layer_body(lp, lpp, g, slice2, table2, 2)

            nc.gpsimd.collective_compute(
                "AllGather", OP.bypass, replica_groups=rg,
                ins=[slice3[:, :]], outs=[table3[:, :]])

            # ---------------- user segment sums + MLP ----------------
            def user_body(up, upp, uw):
                umf = up.tile([128, KU], BF16, tag="umf")
                nc.sync.dma_start(out=umf[:], in_=umeta_p[ds(uw * 128, 128), :])
                uro = up.tile([128, KU], I32, tag="uro")
                nc.sync.dma_start(out=uro[:], in_=urow_p[ds(uw * 128, 128), :])
                ug = up.tile([128, KU, 64], BF16, tag="ug")
                for k in range(KU):
                    nc.gpsimd.indirect_dma_start(
                        out=ug[:, k, :], out_offset=None,
                        in_=table3[:, :],
                        in_offset=bass.IndirectOffsetOnAxis(ap=uro[:, k:k + 1],
                                                            axis=0))
                m01u = up.tile([128, KU, 128], BF16, tag="m01u")
                nc.vector.tensor_tensor(
                    out=m01u[:],
                    in0=_fap(umf[:], [[1, KU], [0, 128]]),
                    in1=_fap(ws["iota128"][:], [[0, KU], [1, 128]]),
                    op=OP.is_equal)
                psy = upp.tile([128, 64], F32, tag="psy")
                for k in range(KU):
                    nc.tensor.matmul(out=psy[:], lhsT=m01u[:, k, :],
                                     rhs=ug[:, k, :],
                                     start=(k == 0), stop=(k == KU - 1))
                ys = up.tile([128, 64], F32, tag="ys")
                nc.scalar.copy(out=ys[:], in_=psy[:])
                ytp = upp.tile([64, 128], F32, tag="ytp")
                nc.tensor.transpose(out=ytp[:], in_=ys[:], identity=ws["ident"][:, :])
                yts = up.tile([64, 128], F32, tag="yts")
                nc.scalar.copy(out=yts[:], in_=ytp[:])
                h1p = upp.tile([32, 128], F32, tag="h1p")
                nc.tensor.matmul(out=h1p[:], lhsT=ws["wf1"][:], rhs=yts[:],
                                 start=True, stop=True)
                h1b = up.tile([32, 128], F32, tag="h1b")
                nc.scalar.activation(out=h1b[:], in_=h1p[:], func=AF.Identity,
                                     bias=ws["bf1"][:, 0:1])
                nc.vector.scalar_tensor_tensor(out=h1b[:], in0=h1b[:],
                                               scalar=LRELU_SLOPE, in1=h1b[:],
                                               op0=OP.mult, op1=OP.max)
                o2p = upp.tile([2, 128], F32, tag="o2p")
                nc.tensor.matmul(out=o2p[:], lhsT=ws["wlab"][:], rhs=h1b[:],
                                 start=True, stop=True)
                o2s = up.tile([2, 128], F32, tag="o2s")
                nc.scalar.activation(out=o2s[:], in_=o2p[:], func=AF.Identity,
                                     bias=ws["blab"][:, 0:1])
                nc.sync.dma_start(out=out_p[:, ds(uw * 128, 128)], in_=o2s[:])

            with tc.tile_pool(name="usr", bufs=2) as up, \
                 tc.tile_pool(name="usrps", bufs=2, space="PSUM") as upp:
                with tc.For_i(0, c.UW, 1) as uw:
                    user_body(up, upp, uw)

    nc.finalize()
    return nc


# --------------------------------------------------------------------------
# Entry point
# --------------------------------------------------------------------------

_CACHE = {}


def _prog(cfg, T, KU):
    key = (cfg.N, cfg.U, T, KU)
    if key not in _CACHE:
        _CACHE[key] = build_program(cfg, T, KU)
    return _CACHE[key]


def make_in_maps(inputs, cfg, pre=None):
    c = cfg
    if pre is None:
        pre = preprocess(inputs, cfg)
    wts = make_weights(inputs, cfg)
    maps = []
    for cc in range(c.C):
        m = {"featT": pre["featT"][cc], "meta": pre["meta"][cc],
             "erow": pre["erow"][cc], "umeta": pre["umeta"][cc],
             "urow": pre["urow"][cc]}
        m.update(wts)
        maps.append(m)
    return pre, maps


def run_all(inputs, cfg, runner):
    pre, maps = make_in_maps(inputs, cfg)
    nc = _prog(cfg, pre["T"], pre["KU"])
    res = runner(nc, maps)
    out = np.zeros((cfg.U, 2), np.float32)
    for cc in range(cfg.C):
        out[cc * cfg.UPCU:(cc + 1) * cfg.UPCU, :] = \
            res[cc]["out"][:, :cfg.UPCU].T
    return out


def kernel(**inputs):
    import jax
    try:
        jax.config.update("jax_compilation_cache_dir",
                          os.environ["JAX_COMPILATION_CACHE_DIR"])
        jax.config.update("jax_persistent_cache_min_entry_size_bytes", -1)
        jax.config.update("jax_persistent_cache_min_compile_time_secs", 0)
    except Exception:
        pass
    from concourse.bass_utils import run_bass_kernel_spmd
    cfg = Cfg()

    def runner(nc, in_maps):
        return run_bass_kernel_spmd(nc, in_maps,
                                    core_ids=list(range(cfg.C))).results

    return run_all(inputs, cfg, runner)


# revision 13
# speedup vs baseline: 1.2961x; 1.2961x over previous
"""BurstGNN Trainium2 kernel — single SPMD launch on 8 NeuronCores.

Graph partitioning per the sharding hint: nodes/edges are partitioned by dst
across the 8 cores; the small weights are replicated. All heavy compute runs
on device in ONE launch:

  encoder (feature-major matmuls) -> per-core [x | al | ar] slice
  -> AllGather (halo exchange)    -> full node table in DRAM (bf16)
  -> FAConv layer 1: per-edge source rows fetched with indirect-DMA gathers
     from the table, alpha = tanh(al_src + ar_dst) * norm computed on device,
     scatter-sum via one-hot selection matmuls accumulating in PSUM
  -> AllGather x1 -> FAConv layer 2 -> |.|-smoothing -> AllGather x2
  -> ragged per-user segment sums (one-hot matmuls) -> 2-layer MLP -> logits.

The host only slots edges/users into fixed-depth windows (pure numpy index
arithmetic) and uploads ~4.5 MB per core, instead of pre-gathering ~100 MB
per core of edge streams. Everything crossing cores moves over NeuronLink.
"""

import os
import sys

sys.path.insert(0, "/opt/trn_rl_repo")
os.environ.setdefault("JAX_COMPILATION_CACHE_DIR", "/tmp/jax_cache")

import ml_dtypes
import numpy as np

import concourse.bass as bass
import concourse.bacc as bacc
import concourse.mybir as mybir
import concourse.tile as tile
from concourse.bass import ds

F32 = mybir.dt.float32
BF16 = mybir.dt.bfloat16
I32 = mybir.dt.int32
AF = mybir.ActivationFunctionType
OP = mybir.AluOpType
AX = mybir.AxisListType

EPS = 0.1
LRELU_SLOPE = 0.01


class Cfg:
    def __init__(self, N=200000, E=1600000, U=20000, NUMP=20, CATP=12):
        self.N, self.E, self.U = N, E, U
        self.NUMP, self.CATP = NUMP, CATP
        self.F = NUMP + CATP
        self.C = 8
        self.D = 64
        self.WJ = 32          # dst window width (one-hot width)
        self.GRP = 16         # windows per group -> 512 dst rows / group
        self.NS = N // self.C
        span = self.WJ * self.GRP
        self.NSP = ((self.NS + span - 1) // span) * span
        self.W = self.NSP // self.WJ
        self.G = self.W // self.GRP
        self.TBLR = self.C * self.NSP
        self.UPCU = U // self.C
        self.UW = (self.UPCU + 127) // 128
        self.UPC = self.UW * 128


def _fap(base, dims, extra_off=0):
    return bass.AP(base.tensor, base.offset + extra_off,
                   [list(base.ap[0])] + [list(d) for d in dims])


# --------------------------------------------------------------------------
# Host preprocessing: edge/user slotting (index arithmetic only)
# --------------------------------------------------------------------------

def preprocess(inputs, cfg):
    c = cfg
    src = np.asarray(inputs["edge_index"][0], dtype=np.int64)
    dst = np.asarray(inputs["edge_index"][1], dtype=np.int64)
    offs = np.asarray(inputs["tweet_offsets"], dtype=np.int64)
    re_index = np.asarray(inputs["re_index"], dtype=np.int64)

    deg = np.bincount(dst, minlength=c.N).astype(np.float64) + 1.0
    dinv = (deg ** -0.5).astype(np.float32)

    srcA = np.concatenate([src, np.arange(c.N, dtype=np.int64)])
    dstA = np.concatenate([dst, np.arange(c.N, dtype=np.int64)])

    core = dstA // c.NS
    dl = dstA - core * c.NS
    wloc = dl // c.WJ
    jloc = (dl - wloc * c.WJ).astype(np.float32)
    gwin = core * c.W + wloc

    cnt = np.bincount(gwin, minlength=c.C * c.W)
    T = max(1, int(-(-cnt.max() // 128)))
    K = c.GRP * T

    order = np.argsort(gwin, kind="stable")
    starts = np.zeros(c.C * c.W + 1, np.int64)
    np.cumsum(cnt, out=starts[1:])
    ranks = np.arange(len(gwin), dtype=np.int64) - starts[gwin[order]]
    t_ = ranks // 128
    p_ = ranks - t_ * 128
    wo = wloc[order]
    k_ = (wo % c.GRP) * T + t_
    g_ = wo // c.GRP
    co = core[order]
    flat = (g_ * 128 + p_) * K + k_

    sz = c.G * 128 * K
    jl8 = np.full((c.C, sz), 255, np.uint8)        # 255 -> one-hot never fires
    # pads gather the (all-zero) last pad row of core C-1
    slot_row = np.full((c.C, sz), c.TBLR - 1, np.int32)
    rowidx = ((srcA // c.NS) * c.NSP + (srcA % c.NS)).astype(np.int32)
    jl8[co, flat] = jloc[order].astype(np.uint8)
    slot_row[co, flat] = rowidx[order]
    jl8 = jl8.reshape(c.C, c.G * 128, K)
    erow = slot_row.reshape(c.C, c.G * 128, K)     # [C, G*128, K]

    # per-node dinv column (pad rows 0)
    dinv_col = np.zeros((c.C, c.NSP, 1), np.float32)
    for cc in range(c.C):
        dinv_col[cc, :c.NS, 0] = dinv[cc * c.NS:(cc + 1) * c.NS]

    # ---- user phase ----
    st = offs[re_index]
    ln = (offs[re_index + 1] - st).astype(np.int64)
    tot = int(ln.sum())
    uu = np.repeat(np.arange(c.U, dtype=np.int64), ln)
    csl = np.cumsum(ln) - ln
    pos = np.arange(tot, dtype=np.int64) - np.repeat(csl, ln)
    nodes = np.repeat(st, ln) + pos
    ucore = uu // c.UPCU
    ulocal = uu - ucore * c.UPCU
    uw = ulocal // 128
    uj = (ulocal - uw * 128).astype(np.float32)
    guw = ucore * c.UW + uw
    ucnt = np.bincount(guw, minlength=c.C * c.UW)
    KU = max(1, int(-(-ucnt.max() // 128)))

    ustarts = np.zeros(c.C * c.UW + 1, np.int64)
    np.cumsum(ucnt, out=ustarts[1:])
    uranks = np.arange(tot, dtype=np.int64) - ustarts[guw]
    ut = uranks // 128
    up = uranks - ut * 128
    uflat = (uw * 128 + up) * KU + ut

    usz = c.UW * 128 * KU
    umeta = np.full((c.C, usz), -1.0, np.float32)
    urow = np.full((c.C, usz), c.TBLR - 1, np.int32)
    urowidx = ((nodes // c.NS) * c.NSP + (nodes % c.NS)).astype(np.int32)
    umeta[ucore, uflat] = uj
    urow[ucore, uflat] = urowidx
    umeta = umeta.reshape(c.C, c.UW * 128, KU)
    urow = urow.reshape(c.C, c.UW * 128, KU)

    # ---- transposed features per core (pad cols zero) ----
    feat = np.concatenate([np.asarray(inputs["num_prop"], np.float32),
                           np.asarray(inputs["cat_prop"], np.float32)], axis=1)
    featT = np.zeros((c.C, c.F, c.NSP), np.float32)
    for cc in range(c.C):
        featT[cc, :, :c.NS] = feat[cc * c.NS:(cc + 1) * c.NS].T

    bf = ml_dtypes.bfloat16
    return dict(T=T, K=K, KU=KU, jl8=jl8, dinv=dinv_col.astype(bf), erow=erow,
                umeta=umeta.astype(bf), urow=urow, featT=featT.astype(bf))


def make_weights(inputs, cfg):
    c = cfg
    f32 = lambda k: np.asarray(inputs[k], np.float32)
    w1 = np.zeros((c.F, 64), np.float32)
    w1[:c.NUMP, :32] = f32("W_num")
    w1[c.NUMP:, 32:] = f32("W_cat")
    b1 = np.concatenate([f32("b_num"), f32("b_cat")]).reshape(64, 1)
    attlr = np.stack([f32("att_l"), f32("att_r")], axis=1)          # [64, 2]
    attB = np.tile(np.concatenate([f32("att_l"), f32("att_r")])[None, :],
                   (128, 1))                                        # [128, 128]
    bf = ml_dtypes.bfloat16
    return {
        "w1": np.ascontiguousarray(w1).astype(bf),
        "b1": np.ascontiguousarray(b1),
        "wtog": f32("W_tog").astype(bf),
        "btog": f32("b_tog").reshape(64, 1),
        "attlr": np.ascontiguousarray(attlr).astype(bf),
        "attB": np.ascontiguousarray(attB),
        "wf1": f32("W_f1"),
        "bf1": f32("b_f1").reshape(32, 1),
        "wlab": f32("W_lab"),
        "blab": f32("b_lab").reshape(2, 1),
        "iota32": np.tile(np.arange(32, dtype=bf)[None, :], (128, 1)),
        "iota128": np.tile(np.arange(128, dtype=np.float32)[None, :],
                           (128, 1)).astype(bf),
        "ident": np.eye(128, dtype=np.float32),
    }


# --------------------------------------------------------------------------
# Bass program
# --------------------------------------------------------------------------

def build_program(cfg, T, KU):
    c = cfg
    K = c.GRP * T
    T4 = 4 * T
    nc = bacc.Bacc()

    U8 = mybir.dt.uint8
    featT_p = nc.declare_dram_parameter("featT", [c.F, c.NSP], BF16, isOutput=False)
    jl8_p = nc.declare_dram_parameter("jl8", [c.G * 128, K], U8, isOutput=False)
    dinv_p = nc.declare_dram_parameter("dinv", [c.NSP, 1], BF16, isOutput=False)
    erow_p = nc.declare_dram_parameter("erow", [c.G * 128, K], I32, isOutput=False)
    umeta_p = nc.declare_dram_parameter("umeta", [c.UW * 128, KU], BF16, isOutput=False)
    urow_p = nc.declare_dram_parameter("urow", [c.UW * 128, KU], I32, isOutput=False)
    wparams = {}
    BF_W = {"w1", "wtog", "attlr", "iota32", "iota128"}
    for name, shape in [("w1", [c.F, 64]), ("b1", [64, 1]),
                        ("wtog", [64, 64]), ("btog", [64, 1]),
                        ("attlr", [64, 2]), ("attB", [128, 128]),
                        ("wf1", [64, 32]), ("bf1", [32, 1]),
                        ("wlab", [32, 2]), ("blab", [2, 1]),
                        ("iota32", [128, 32]), ("iota128", [128, 128]),
                        ("ident", [128, 128])]:
        wparams[name] = nc.declare_dram_parameter(
            name, shape, BF16 if name in BF_W else F32, isOutput=False)
    out_p = nc.declare_dram_parameter("out", [2, c.UPC], F32, isOutput=True)

    slice1 = nc.dram_tensor("slice1", [c.NSP, 68], BF16)
    slice2 = nc.dram_tensor("slice2", [c.NSP, 68], BF16)
    slice3 = nc.dram_tensor("slice3", [c.NSP, 64], BF16)
    table1 = nc.dram_tensor("table1", [c.TBLR, 68], BF16, addr_space="Shared")
    table2 = nc.dram_tensor("table2", [c.TBLR, 68], BF16, addr_space="Shared")
    table3 = nc.dram_tensor("table3", [c.TBLR, 64], BF16, addr_space="Shared")

    rg = [list(range(c.C))]

    with tile.TileContext(nc) as tc:
        with tc.tile_pool(name="consts", bufs=1) as cp:
            ws = {}
            for name, p in wparams.items():
                t = cp.tile(list(p.shape), BF16 if name in BF_W else F32, tag=name)
                nc.sync.dma_start(out=t[:], in_=p[:, :])
                ws[name] = t
            ones1 = cp.tile([1, 128], F32)
            nc.vector.memset(ones1[:], 1.0)
            identb = cp.tile([128, 128], BF16)
            nc.vector.tensor_copy(out=identb[:], in_=ws["ident"][:, :])
            beps = cp.tile([128, 1], F32)
            nc.vector.memset(beps[:], 1e-8)
            la = cp.tile([128, K, 128], BF16)
            nc.vector.memset(la[:].rearrange("p k f -> p (k f)"), 0.0)

            # ---------------- encoder ----------------
            def enc_body(ep, epp, t0, pad_from=None):
                ft = ep.tile([c.F, 512], BF16, tag="ft")
                nc.sync.dma_start(out=ft[:], in_=featT_p[:, ds(t0, 512)])
                ps1 = epp.tile([64, 512], F32, tag="ps1")
                nc.tensor.matmul(out=ps1[:], lhsT=ws["w1"][:], rhs=ft[:],
                                 start=True, stop=True)
                mid = ep.tile([64, 512], BF16, tag="mid")
                nc.scalar.activation(out=mid[:], in_=ps1[:], func=AF.Identity,
                                     bias=ws["b1"][:, 0:1])
                nc.vector.scalar_tensor_tensor(out=mid[:], in0=mid[:],
                                               scalar=LRELU_SLOPE, in1=mid[:],
                                               op0=OP.mult, op1=OP.max)
                ps2 = epp.tile([64, 512], F32, tag="ps2")
                nc.tensor.matmul(out=ps2[:], lhsT=ws["wtog"][:], rhs=mid[:],
                                 start=True, stop=True)
                xc = ep.tile([66, 512], BF16, tag="xc")
                nc.scalar.activation(out=xc[0:64, :], in_=ps2[:], func=AF.Identity,
                                     bias=ws["btog"][:, 0:1])
                nc.vector.scalar_tensor_tensor(out=xc[0:64, :], in0=xc[0:64, :],
                                               scalar=LRELU_SLOPE, in1=xc[0:64, :],
                                               op0=OP.mult, op1=OP.max)
                if pad_from is not None and pad_from < 512:
                    nc.vector.memset(xc[0:64, pad_from:512], 0.0)
                ps3 = epp.tile([2, 512], F32, tag="ps3")
                nc.tensor.matmul(out=ps3[:], lhsT=ws["attlr"][:], rhs=xc[0:64, :],
                                 start=True, stop=True)
                nc.scalar.copy(out=xc[64:66, :], in_=ps3[:])
                if pad_from is not None and pad_from < 512:
                    nc.vector.memset(xc[64:66, pad_from:512], 0.0)
                for cc4 in range(4):
                    tp = epp.tile([128, 66], BF16, tag="tp")
                    nc.tensor.transpose(out=tp[:], in_=xc[:, cc4 * 128:(cc4 + 1) * 128],
                                        identity=identb[0:66, 0:66])
                    pb = ep.tile([128, 68], BF16, tag="pb")
                    nc.scalar.copy(out=pb[:, 0:66], in_=tp[:])
                    dv = ep.tile([128, 1], BF16, tag="dv")
                    nc.sync.dma_start(out=dv[:],
                                      in_=dinv_p[ds(t0 + cc4 * 128, 128), :])
                    nc.scalar.copy(out=pb[:, 66:67], in_=dv[:])
                    nc.vector.memset(pb[:, 67:68], 0.0)
                    nc.sync.dma_start(out=slice1[ds(t0 + cc4 * 128, 128), :],
                                      in_=pb[:])

            with tc.tile_pool(name="enc", bufs=2) as ep, \
                 tc.tile_pool(name="encps", bufs=1, space="PSUM") as epp:
                n_full = c.NSP // 512 - 1
                if n_full > 0:
                    with tc.For_i(0, n_full * 512, 512) as t0:
                        enc_body(ep, epp, t0)
                last0 = n_full * 512
                enc_body(ep, epp, last0, pad_from=c.NS - last0)

            nc.gpsimd.collective_compute(
                "AllGather", OP.bypass, replica_groups=rg,
                ins=[slice1[:, :]], outs=[table1[:, :]])

            # ---------------- FAConv layers ----------------
            def layer_body(lp, lpp, g, src_slice, src_table, layer):
                jl8t = lp.tile([128, K], mybir.dt.uint8, tag="jl8t")
                nc.sync.dma_start(out=jl8t[:], in_=jl8_p[ds(g * 128, 128), :])
                jlb = lp.tile([128, K], BF16, tag="jlb")
                nc.vector.tensor_copy(out=jlb[:], in_=jl8t[:])
                ger = lp.tile([128, K], I32, tag="ger")
                nc.sync.dma_start(out=ger[:], in_=erow_p[ds(g * 128, 128), :])
                hg = lp.tile([128, K, 68], BF16, tag="hg")
                for k in range(K):
                    nc.gpsimd.indirect_dma_start(
                        out=hg[:, k, :], out_offset=None,
                        in_=src_table[:, :],
                        in_offset=bass.IndirectOffsetOnAxis(ap=ger[:, k:k + 1],
                                                            axis=0))
                for h in range(4):
                    rb = g * 512 + h * 128
                    kb = h * T4
                    x0b = lp.tile([128, 64], BF16, tag="x0b")
                    nc.sync.dma_start(out=x0b[:], in_=slice1[ds(rb, 128), 0:64])
                    arc = lp.tile([128, 1], BF16, tag="arc")
                    nc.sync.dma_start(out=arc[:], in_=src_slice[ds(rb, 128), 65:66])
                    dvc = lp.tile([128, 1], BF16, tag="dvc")
                    nc.sync.dma_start(out=dvc[:], in_=slice1[ds(rb, 128), 66:67])
                    artp = lpp.tile([1, 128], BF16, tag="artp")
                    nc.tensor.transpose(out=artp[:], in_=arc[:],
                                        identity=identb[:])
                    arsh = lp.tile([1, 128], F32, tag="arsh")
                    nc.scalar.copy(out=arsh[:], in_=artp[:])
                    arw = lpp.tile([128, T4 * 32], F32, tag="arw")
                    nc.tensor.matmul(
                        out=arw[:], lhsT=ones1[:],
                        rhs=_fap(arsh[:], [[32, 4], [0, T], [1, 32]]),
                        start=True, stop=True)
                    dvtp = lpp.tile([1, 128], BF16, tag="dvtp")
                    nc.tensor.transpose(out=dvtp[:], in_=dvc[:],
                                        identity=identb[:])
                    dvsh = lp.tile([1, 128], F32, tag="dvsh")
                    nc.scalar.copy(out=dvsh[:], in_=dvtp[:])
                    dvw = lpp.tile([128, T4 * 32], F32, tag="dvw")
                    nc.tensor.matmul(
                        out=dvw[:], lhsT=ones1[:],
                        rhs=_fap(dvsh[:], [[32, 4], [0, T], [1, 32]]),
                        start=True, stop=True)
                    addt = lp.tile([128, T4 * 32], F32, tag="addt")
                    nc.vector.tensor_tensor(
                        out=addt[:],
                        in0=_fap(hg[:].rearrange("p k f -> p (k f)"),
                                 [[68, T4], [0, 32]], extra_off=kb * 68 + 64),
                        in1=arw[:], op=OP.add)
                    tanhb = lp.tile([128, T4 * 32], BF16, tag="tanhb")
                    nc.scalar.activation(out=tanhb[:], in_=addt[:], func=AF.Tanh)
                    alph = lp.tile([128, T4 * 32], BF16, tag="alph")
                    nc.vector.tensor_tensor(
                        out=alph[:], in0=tanhb[:],
                        in1=_fap(hg[:].rearrange("p k f -> p (k f)"),
                                 [[68, T4], [0, 32]], extra_off=kb * 68 + 66),
                        op=OP.mult)
                    m01 = lp.tile([128, T4 * 32], BF16, tag="m01")
                    nc.vector.tensor_tensor(
                        out=m01[:],
                        in0=_fap(jlb[:], [[1, T4], [0, 32]], extra_off=kb),
                        in1=_fap(ws["iota32"][:], [[0, T4], [1, 32]]),
                        op=OP.is_equal)
                    m2 = lp.tile([128, T4 * 32], BF16, tag="m2")
                    nc.vector.tensor_tensor(
                        out=m2[:], in0=m01[:], in1=dvw[:], op=OP.mult)
                    nc.vector.tensor_tensor(
                        out=_fap(la[:].rearrange("p k f -> p (k f)"),
                                 [[T * 128 + 32, 4], [128, T], [1, 32]],
                                 extra_off=kb * 128),
                        in0=m2[:], in1=alph[:], op=OP.mult)
                    ps = lpp.tile([128, 64], F32, tag="ps")
                    for kk in range(T4):
                        k = kb + kk
                        nc.tensor.matmul(out=ps[:], lhsT=la[:, k, :],
                                         rhs=hg[:, k, 0:64],
                                         start=(kk == 0), stop=(kk == T4 - 1))
                    xo = lp.tile([128, 64], F32, tag="xo")
                    nc.vector.scalar_tensor_tensor(
                        out=xo[:], in0=x0b[:], scalar=EPS, in1=ps[:],
                        op0=OP.mult, op1=OP.add)
                    if layer == 1:
                        pack = lp.tile([128, 68], BF16, tag="pack")
                        nc.scalar.copy(out=pack[:, 0:64], in_=xo[:])
                        nc.scalar.copy(out=pack[:, 66:67], in_=dvc[:])
                        nc.vector.memset(pack[:, 67:68], 0.0)
                        palr = lp.tile([128, 2], F32, tag="palr")
                        tsc = lp.tile([128, 64], F32, tag="tsc")
                        nc.vector.tensor_tensor(out=tsc[:], in0=xo[:],
                                                in1=ws["attB"][:, 0:64], op=OP.mult)
                        nc.vector.tensor_reduce(out=palr[:, 0:1], in_=tsc[:],
                                                axis=AX.X, op=OP.add)
                        tsc2 = lp.tile([128, 64], F32, tag="tsc2")
                        nc.vector.tensor_tensor(out=tsc2[:], in0=xo[:],
                                                in1=ws["attB"][:, 64:128], op=OP.mult)
                        nc.vector.tensor_reduce(out=palr[:, 1:2], in_=tsc2[:],
                                                axis=AX.X, op=OP.add)
                        nc.scalar.copy(out=pack[:, 64:66], in_=palr[:])
                        nc.sync.dma_start(out=slice2[ds(rb, 128), :], in_=pack[:])
                    else:
                        xs = lp.tile([128, 64], F32, tag="xs")
                        nc.scalar.activation(out=xs[:], in_=xo[:], func=AF.Square)
                        pack2 = lp.tile([128, 64], BF16, tag="pack2")
                        nc.scalar.activation(out=pack2[:], in_=xs[:], func=AF.Sqrt,
                                             bias=beps[:, 0:1])
                        nc.sync.dma_start(out=slice3[ds(rb, 128), :], in_=pack2[:])

            with tc.tile_pool(name="lay1", bufs=2) as lp, \
                 tc.tile_pool(name="lay1ps", bufs=1, space="PSUM") as lpp:
                with tc.For_i(0, c.G, 1) as g:
                    layer_body(lp, lpp, g, slice1, table1, 1)

            nc.gpsimd.collective_compute(
                "AllGather", OP.bypass, replica_groups=rg,
                ins=[slice2[:, :]], outs=[table2[:, :]])

            with tc.tile_pool(name="lay2", bufs=2) as lp, \
                 tc.tile_pool(name="lay2ps", bufs=1, space="PSUM") as lpp:
                with tc.For_i(0, c.G, 1) as g:
                    layer_body(lp, lpp, g, slice2, table2, 2)

            nc.gpsimd.collective_compute(
                "AllGather", OP.bypass, replica_groups=rg,
                ins=[slice3[:, :]], outs=[table3[:, :]])

            # ---------------- user segment sums + MLP ----------------
            def user_body(up, upp, uw):
                umf = up.tile([128, KU], BF16, tag="umf")
                nc.sync.dma_start(out=umf[:], in_=umeta_p[ds(uw * 128, 128), :])
                uro = up.tile([128, KU], I32, tag="uro")
                nc.sync.dma_start(out=uro[:], in_=urow_p[ds(uw * 128, 128), :])
                ug = up.tile([128, KU, 64], BF16, tag="ug")
                for k in range(KU):
                    nc.gpsimd.indirect_dma_start(
                        out=ug[:, k, :], out_offset=None,
                        in_=table3[:, :],
                        in_offset=bass.IndirectOffsetOnAxis(ap=uro[:, k:k + 1],
                                                            axis=0))
                m01u = up.tile([128, KU, 128], BF16, tag="m01u")
                nc.vector.tensor_tensor(
                    out=m01u[:],
                    in0=_fap(umf[:], [[1, KU], [0, 128]]),
                    in1=_fap(ws["iota128"][:], [[0, KU], [1, 128]]),
                    op=OP.is_equal)
                psy = upp.tile([128, 64], F32, tag="psy")
                for k in range(KU):
                    nc.tensor.matmul(out=psy[:], lhsT=m01u[:, k, :],
                                     rhs=ug[:, k, :],
                                     start=(k == 0), stop=(k == KU - 1))
                ys = up.tile([128, 64], F32, tag="ys")
                nc.scalar.copy(out=ys[:], in_=psy[:])
                ytp = upp.tile([64, 128], F32, tag="ytp")
                nc.tensor.transpose(out=ytp[:], in_=ys[:], identity=ws["ident"][:, :])
                yts = up.tile([64, 128], F32, tag="yts")
                nc.scalar.copy(out=yts[:], in_=ytp[:])
                h1p = upp.tile([32, 128], F32, tag="h1p")
                nc.tensor.matmul(out=h1p[:], lhsT=ws["wf1"][:], rhs=yts[:],
                                 start=True, stop=True)
                h1b = up.tile([32, 128], F32, tag="h1b")
                nc.scalar.activation(out=h1b[:], in_=h1p[:], func=AF.Identity,
                                     bias=ws["bf1"][:, 0:1])
                nc.vector.scalar_tensor_tensor(out=h1b[:], in0=h1b[:],
                                               scalar=LRELU_SLOPE, in1=h1b[:],
                                               op0=OP.mult, op1=OP.max)
                o2p = upp.tile([2, 128], F32, tag="o2p")
                nc.tensor.matmul(out=o2p[:], lhsT=ws["wlab"][:], rhs=h1b[:],
                                 start=True, stop=True)
                o2s = up.tile([2, 128], F32, tag="o2s")
                nc.scalar.activation(out=o2s[:], in_=o2p[:], func=AF.Identity,
                                     bias=ws["blab"][:, 0:1])
                nc.sync.dma_start(out=out_p[:, ds(uw * 128, 128)], in_=o2s[:])

            with tc.tile_pool(name="usr", bufs=2) as up, \
                 tc.tile_pool(name="usrps", bufs=2, space="PSUM") as upp:
                with tc.For_i(0, c.UW, 1) as uw:
                    user_body(up, upp, uw)

    nc.finalize()
    return nc


# --------------------------------------------------------------------------
# Entry point
# --------------------------------------------------------------------------

_CACHE = {}


def _prog(cfg, T, KU):
    key = (cfg.N, cfg.U, T, KU)
    if key not in _CACHE:
        _CACHE[key] = build_program(cfg, T, KU)
    return _CACHE[key]


def make_in_maps(inputs, cfg, pre=None):
    c = cfg
    if pre is None:
        pre = preprocess(inputs, cfg)
    wts = make_weights(inputs, cfg)
    maps = []
    for cc in range(c.C):
        m = {"featT": pre["featT"][cc], "jl8": pre["jl8"][cc],
             "dinv": pre["dinv"][cc], "erow": pre["erow"][cc],
             "umeta": pre["umeta"][cc], "urow": pre["urow"][cc]}
        m.update(wts)
        maps.append(m)
    return pre, maps


def run_all(inputs, cfg, runner):
    pre, maps = make_in_maps(inputs, cfg)
    nc = _prog(cfg, pre["T"], pre["KU"])
    res = runner(nc, maps)
    out = np.zeros((cfg.U, 2), np.float32)
    for cc in range(cfg.C):
        out[cc * cfg.UPCU:(cc + 1) * cfg.UPCU, :] = \
            res[cc]["out"][:, :cfg.UPCU].T
    return out


def kernel(**inputs):
    import jax
    try:
        jax.config.update("jax_compilation_cache_dir",
                          os.environ["JAX_COMPILATION_CACHE_DIR"])
        jax.config.update("jax_persistent_cache_min_entry_size_bytes", -1)
        jax.config.update("jax_persistent_cache_min_compile_time_secs", 0)
    except Exception:
        pass
    from concourse.bass_utils import run_bass_kernel_spmd
    cfg = Cfg()

    def runner(nc, in_maps):
        return run_bass_kernel_spmd(nc, in_maps,
                                    core_ids=list(range(cfg.C))).results

    return run_all(inputs, cfg, runner)
